# revision 22
# baseline (speedup 1.0000x reference)
"""ColBERT pairwise + in-batch negative CE loss on 8 Trainium2 NeuronCores.

Problem shapes (hardcoded): B=64, N=32, S=256, D=128, fp32.

reference:
    pos_scores[b]  = sum_n max_s  q[b,n,:] . d[b,s,:]
    neg_scores[b]  = sum_n max_s  q[b,n,:] . neg[b,s,:]
    scores[b,c]    = sum_n max_s  q[b,n,:] . d[c,s,:]
    loss = (mean softplus(neg_scores - pos_scores)
            + mean softplus(max_offdiag_c scores[b,c] - scores[b,b])) / 2

Sharding: the in-batch score matrix is sharded over the doc dim c (8 docs per
core; every core sees all 64*32 query rows).  The pairwise-neg term is
data-parallel over b (8 queries + their neg docs per core).  The host
pre-transposes all operands to d-major layout so the device does zero
transposes; the contraction dim d=128 maps exactly onto the PE partition dim.

Per core the device produces a (4, 130) fp32 tile:
  cols 0..127:  col 8*m+c, row j  ->  sum_n max_s (q[4m+j] . d_local[c])
  cols 128/129: col 128+g, row j  ->  neg_scores for local b = 4g+j
The host assembles the full (64, 64) scores matrix + the 64 neg pairwise
scores and applies the trivial softplus/mean epilogue (128 scalars).
"""

import sys

import numpy as np


def _ensure_path():
    try:
        import concourse  # noqa: F401
    except ImportError:
        sys.path.insert(0, "/opt/trn_rl_repo")


_ensure_path()

import concourse.bacc as bacc  # noqa: E402
import concourse.mybir as mybir  # noqa: E402
from concourse.bass_utils import run_bass_kernel_spmd  # noqa: E402
from concourse.tile import TileContext  # noqa: E402

B, N, S, D = 64, 32, 256, 128
NC = 8
CL = B // NC  # docs / queries per core (8)
BN = B * N  # 2048 query rows
DCOLS = CL * S  # 2048 doc columns per core
NEG_INF_DIAG = 1000000.0

F32 = mybir.dt.float32
F16 = mybir.dt.float16
MMDT = mybir.dt.float16  # dtype used by the matmul operands

_CACHE = {}


def _install_ntff_shim():
    """Best-effort: register the axon NTFF profile hook so BASS_TRACE=1
    produces hardware profiles.  Safe no-op when unavailable."""
    try:
        import types

        import antenv

        if "antenv.axon_hooks" in sys.modules:
            return
        import trn_agent_boot.trn_boot as tb

        mod = types.ModuleType("antenv.axon_hooks")
        _hook = [None]
        mod.set_axon_ntff_profile_hook = lambda h: _hook.__setitem__(0, h)
        mod.get_axon_ntff_profile_hook = lambda: _hook[0]
        sys.modules["antenv.axon_hooks"] = mod
        antenv.axon_hooks = mod
        mod.set_axon_ntff_profile_hook(
            tb._ntff_profile_via_ctypes("/opt/axon/libaxon_pjrt.so")
        )
    except Exception:
        pass


def _build():
    nc = bacc.Bacc("TRN2", target_bir_lowering=False, debug=False, num_devices=NC)
    qT = nc.dram_tensor("qT", [D, BN], MMDT, kind="ExternalInput")
    dT = nc.dram_tensor("dT", [D, DCOLS], MMDT, kind="ExternalInput")
    nT = nc.dram_tensor("nT", [D, DCOLS], MMDT, kind="ExternalInput")
    qp = nc.dram_tensor("qp", [D, CL * N], MMDT, kind="ExternalInput")
    ones = nc.dram_tensor("ones", [D, 4], F16, kind="ExternalInput")
    out_d = nc.dram_tensor("out", [4, 130], F32, kind="ExternalOutput")

    with TileContext(nc) as tc:
        with (
            tc.tile_pool(name="sb", bufs=1) as sb,
            tc.tile_pool(name="ps", bufs=2, space="PSUM") as ps,
        ):
            qs = sb.tile([D, BN], MMDT, tag="qs")
            ds = sb.tile([D, DCOLS], MMDT, tag="ds")
            ns = sb.tile([D, DCOLS], MMDT, tag="ns")
            qps = sb.tile([D, CL * N], MMDT, tag="qps")
            onesb = sb.tile([D, 4], F16, tag="ones")
            maxall = sb.tile([128, 130], F16, tag="maxall")
            outsb = sb.tile([4, 130], F32, tag="outsb")

            # DMA order: graduated piece sizes so chunk m=0 (qs cols 0:128,
            # ds cols 0:1024) can start after ~300KB instead of the full load.
            nc.sync.dma_start(out=qs[:, 0:128], in_=qT[:, 0:128])
            nc.sync.dma_start(out=ds[:, 0:512], in_=dT[:, 0:512])
            nc.sync.dma_start(out=ds[:, 512:1024], in_=dT[:, 512:1024])
            nc.sync.dma_start(out=qs[:, 128:1024], in_=qT[:, 128:1024])
            nc.sync.dma_start(out=ds[:, 1024:2048], in_=dT[:, 1024:2048])
            nc.sync.dma_start(out=qs[:, 1024:2048], in_=qT[:, 1024:2048])
            for p in range(4):
                sl = slice(512 * p, 512 * (p + 1))
                nc.sync.dma_start(out=ns[:, sl], in_=nT[:, sl])
            nc.sync.dma_start(out=qps[:, :], in_=qp[:, :])
            nc.sync.dma_start(out=onesb[:, :], in_=ones[:, :])

            # In-batch term: query chunk m (128 rows) x all 2048 local doc
            # cols.  m=0 is split in two half-width tiles so the first
            # reduce only gates on ds[0:1024]; the rest use full tiles.
            for m in range(16):
                if m == 0:
                    for h in range(2):
                        t = ps.tile([128, 1024], F32, tag="chunk")
                        for u in range(2):
                            c0 = 1024 * h + 512 * u
                            nc.tensor.matmul(
                                t[:, 512 * u : 512 * (u + 1)],
                                qs[:, 0:128],
                                ds[:, c0 : c0 + 512],
                                start=True,
                                stop=True,
                            )
                        nc.vector.reduce_max(
                            maxall[:, 4 * h : 4 * h + 4],
                            t[:, :].rearrange("p (g s) -> p g s", s=S),
                            axis=mybir.AxisListType.X,
                        )
                    continue
                t = ps.tile([128, 2048], F32, tag="chunk")
                for u in range(4):
                    nc.tensor.matmul(
                        t[:, 512 * u : 512 * (u + 1)],
                        qs[:, 128 * m : 128 * (m + 1)],
                        ds[:, 512 * u : 512 * (u + 1)],
                        start=True,
                        stop=True,
                    )
                nc.vector.reduce_max(
                    maxall[:, 8 * m : 8 * m + 8],
                    t[:, :].rearrange("p (g s) -> p g s", s=S),
                    axis=mybir.AxisListType.X,
                )

            # Pairwise neg term: 8 small matmuls (M=32) col-packed 4-way via
            # tile_position; maxes land in maxall[:, 128:130] with local b at
            # partitions 32*(b%4) + n, column 128 + b//4.
            for gidx in range(2):
                pt = ps.tile([128, 256], F32, tag="chunk")
                for j in range(4):
                    b = 4 * gidx + j
                    nc.tensor.matmul(
                        pt[32 * j : 32 * (j + 1), :],
                        qps[:, 32 * b : 32 * (b + 1)],
                        ns[:, 256 * b : 256 * (b + 1)],
                        start=True,
                        stop=True,
                        tile_position=(0, 32 * j),
                    )
                nc.vector.reduce_max(
                    maxall[:, 128 + gidx : 129 + gidx],
                    pt[:, :],
                    axis=mybir.AxisListType.X,
                )

            # n-sum via block-ones matmul: out[j, col] = sum_{n} maxall[32j+n, col]
            # Split at col 64 (chunks m<8 finish first) so the first half of
            # the epilogue overlaps the remaining reduces.
            for c0, c1 in ((0, 64), (64, 130)):
                ot = ps.tile([4, 130], F32, tag="chunk")
                nc.tensor.matmul(
                    ot[:, 0 : c1 - c0],
                    onesb[:, :],
                    maxall[:, c0:c1],
                    start=True,
                    stop=True,
                )
                nc.vector.tensor_copy(outsb[:, c0:c1], ot[:, 0 : c1 - c0])
                nc.sync.dma_start(out=out_d[:, c0:c1], in_=outsb[:, c0:c1])

    nc.finalize()
    return nc


LAST_RESULT = None


def kernel(query_embeddings, doc_embeddings, neg_doc_embeddings):
    global LAST_RESULT
    _install_ntff_shim()

    q = np.asarray(query_embeddings, dtype=np.float32)
    d = np.asarray(doc_embeddings, dtype=np.float32)
    g = np.asarray(neg_doc_embeddings, dtype=np.float32)
    assert q.shape == (B, N, D) and d.shape == (B, S, D) and g.shape == (B, S, D)

    # d-major layouts
    qT_all = np.ascontiguousarray(q.transpose(2, 0, 1).reshape(D, BN).astype(np.float16))
    ones_blk = np.zeros((D, 4), dtype=np.float16)
    ones_blk[np.arange(D), np.arange(D) // 32] = 1.0

    in_maps = []
    for k in range(NC):
        dT_k = np.ascontiguousarray(
            d[CL * k : CL * (k + 1)].transpose(2, 0, 1).reshape(D, DCOLS).astype(np.float16)
        )
        nT_k = np.ascontiguousarray(
            g[CL * k : CL * (k + 1)].transpose(2, 0, 1).reshape(D, DCOLS).astype(np.float16)
        )
        qp_k = np.ascontiguousarray(qT_all[:, CL * N * k : CL * N * (k + 1)])
        in_maps.append(
            {"qT": qT_all, "dT": dT_k, "nT": nT_k, "qp": qp_k, "ones": ones_blk}
        )

    if "nc" not in _CACHE:
        _CACHE["nc"] = _build()
    res = run_bass_kernel_spmd(_CACHE["nc"], in_maps, core_ids=list(range(NC)))
    LAST_RESULT = res

    # Assemble: scores (64, 64) and pairwise neg scores (64,)
    scores = np.empty((B, B), dtype=np.float32)
    negpair = np.empty((B,), dtype=np.float32)
    for k in range(NC):
        o = res.results[k]["out"]  # (4, 130)
        scores[:, CL * k : CL * (k + 1)] = (
            o[:, :128].reshape(4, 16, CL).transpose(1, 0, 2).reshape(B, CL)
        )
        for gcol in range(2):
            for j in range(4):
                negpair[CL * k + 4 * gcol + j] = o[j, 128 + gcol]

    pos = np.diagonal(scores).astype(np.float64)
    l1 = np.logaddexp(0.0, negpair.astype(np.float64) - pos).mean()
    neg_ib = (
        scores.astype(np.float64) - np.eye(B, dtype=np.float64) * NEG_INF_DIAG
    ).max(axis=1)
    l2 = np.logaddexp(0.0, neg_ib - pos).mean()
    return np.asarray((l1 + l2) / 2.0, dtype=np.float32)
